# revision 1
# baseline (speedup 1.0000x reference)
"""ASPPDeformable Trainium2 Bass kernel.

Sharding: 8 cores, core n -> (batch n//4, output rows 16*(n%4) .. +16), i.e.
1024 pixels per core; x (the core's batch) replicated in SBUF as a zero-padded
70x70 flat grid (y,x in [-3..66]) used both for conv im2col views and for the
deformable bilinear gathers (GPSIMD ap_gather).

Per branch: offset/mod convs as PSUM-accumulated matmuls (3 groups: y, x, mod,
each [KK, 1024]); per-pixel floor/frac/validity/corner-weight math on DVE/ACT;
per kernel-position kk: 4 corner gathers, weighted corner combine on DVE (with
DMA-replicated weight rows), main conv matmul accumulating over kk in PSUM;
BN+ReLU fused into the PSUM eviction. Global-avg-pool branch contributes a
pixel-independent column folded into the fuse-conv bias.
"""

import os
import sys
import numpy as np

for _p in ("/opt/trn_rl_repo", "/root/.axon_site/_ro/trn_rl_repo"):
    if os.path.isdir(_p) and _p not in sys.path:
        sys.path.insert(0, _p)

import concourse.bass as bass
import concourse.bacc as bacc
import concourse.mybir as mybir
from concourse import tile
from concourse import library_config
from concourse.bass_utils import run_bass_kernel_spmd

dt = mybir.dt
F32 = dt.float32
I16 = dt.int16
I32 = dt.int32
ALU = mybir.AluOpType
ACTF = mybir.ActivationFunctionType
AXL = mybir.AxisListType

KS = (1, 1, 3, 7)
PADS = (0, 0, 1, 3)
KKS = (1, 1, 9, 49)
B, C, H, W = 2, 256, 64, 64
EPS = 1e-5
MARG = 3
GW = 70                        # grid coords -3..66
GN = GW * GW                   # 4900
GNP = GN + 12                  # 4912, small tail pad
PXC = 1024
RB = 16
NCORES = 8


def _bn_fold(g, b, m, v):
    s = (np.asarray(g) / np.sqrt(np.asarray(v) + EPS)).astype(np.float32)
    t = (np.asarray(b) - np.asarray(m) * s).astype(np.float32)
    return s, t


def build_program():
    nc = bacc.Bacc("TRN2", target_bir_lowering=False, debug=False,
                   num_devices=NCORES)

    xp_d = nc.dram_tensor("xp", [128, 2, GNP], F32, kind="ExternalInput")
    xl_d = nc.dram_tensor("xl", [128, 2, 22, GW], F32, kind="ExternalInput")
    ins = {}
    for br in range(4):
        k, KK = KS[br], KKS[br]
        T = k * k * 2
        ins[f"cby{br}"] = nc.dram_tensor(f"cby{br}", [KK, PXC], F32, kind="ExternalInput")
        ins[f"cbx{br}"] = nc.dram_tensor(f"cbx{br}", [KK, PXC], F32, kind="ExternalInput")
        ins[f"wof{br}"] = nc.dram_tensor(f"wof{br}", [T, 128, 3 * KK], F32, kind="ExternalInput")
        ins[f"bof{br}"] = nc.dram_tensor(f"bof{br}", [KK, 3], F32, kind="ExternalInput")
        ins[f"wm{br}"] = nc.dram_tensor(f"wm{br}", [KK, 2, 2, 128, 128], F32, kind="ExternalInput")
        ins[f"sbn{br}"] = nc.dram_tensor(f"sbn{br}", [128, 2], F32, kind="ExternalInput")
        ins[f"tbn{br}"] = nc.dram_tensor(f"tbn{br}", [128, 2], F32, kind="ExternalInput")
    wfu_d = nc.dram_tensor("wfu", [8, 128, 2, 128], F32, kind="ExternalInput")
    wfp_d = nc.dram_tensor("wfp", [2, 128, 2, 128], F32, kind="ExternalInput")
    s1_d = nc.dram_tensor("s1", [128, 2], F32, kind="ExternalInput")
    t1_d = nc.dram_tensor("t1", [128, 2], F32, kind="ExternalInput")
    wpl_d = nc.dram_tensor("wpl", [2, 128, 2, 128], F32, kind="ExternalInput")
    s5_d = nc.dram_tensor("s5", [128, 2], F32, kind="ExternalInput")
    t5_d = nc.dram_tensor("t5", [128, 2], F32, kind="ExternalInput")
    outp_d = nc.dram_tensor("outp", [2, 128, PXC], F32, kind="ExternalOutput")

    with tile.TileContext(nc) as tc:
        with (
            tc.tile_pool(name="big", bufs=1) as big,
            tc.tile_pool(name="wload", bufs=3) as wload,
            tc.tile_pool(name="mathp", bufs=1) as mathp,
            tc.tile_pool(name="gath", bufs=2) as gath,
            tc.tile_pool(name="wrp", bufs=1) as wrp,
            tc.tile_pool(name="ps", bufs=1, space="PSUM") as ps,
            tc.tile_pool(name="dscr", bufs=1, space="DRAM") as dscr,
        ):
            nc.gpsimd.load_library(library_config.ap_gather)

            xp = big.tile([128, 2, GNP], F32)
            nc.sync.dma_start(xp[:, :, :], xp_d[:, :, :])
            xl = big.tile([128, 2, 22, GW], F32)
            nc.sync.dma_start(xl[:, :, :, :], xl_d[:, :, :, :])

            # ---- pool branch ----
            xm = big.tile([128, 2], F32)
            for h in range(2):
                nc.vector.tensor_reduce(xm[:, h:h + 1], xp[:, h, :],
                                        axis=AXL.X, op=ALU.add)
            nc.vector.tensor_scalar_mul(xm[:, :], xm[:, :], 1.0 / 4096.0)

            wpl = big.tile([128, 2, 2, 128], F32)
            for ch in range(2):
                nc.sync.dma_start(wpl[:, ch, :, :], wpl_d[ch, :, :, :])
            s5 = big.tile([128, 2], F32)
            t5 = big.tile([128, 2], F32)
            nc.sync.dma_start(s5[:, :], s5_d[:, :])
            nc.sync.dma_start(t5[:, :], t5_d[:, :])
            x5 = big.tile([128, 2], F32)
            for oh in range(2):
                pp = ps.tile([128, 1], F32, tag="C")
                for ch in range(2):
                    nc.tensor.matmul(pp[:, :], wpl[:, ch, oh, :], xm[:, ch:ch + 1],
                                     start=(ch == 0), stop=(ch == 1))
                nc.scalar.activation(x5[:, oh:oh + 1], pp[:, :], ACTF.Relu,
                                     bias=t5[:, oh:oh + 1], scale=s5[:, oh:oh + 1])

            wfp = big.tile([128, 2, 2, 128], F32)
            for ch in range(2):
                nc.sync.dma_start(wfp[:, ch, :, :], wfp_d[ch, :, :, :])
            s1 = big.tile([128, 2], F32)
            t1 = big.tile([128, 2], F32)
            nc.sync.dma_start(s1[:, :], s1_d[:, :])
            nc.sync.dma_start(t1[:, :], t1_d[:, :])
            fbias = big.tile([128, 2], F32)
            for oh in range(2):
                pp = ps.tile([128, 1], F32, tag="C")
                for ch in range(2):
                    nc.tensor.matmul(pp[:, :], wfp[:, ch, oh, :], x5[:, ch:ch + 1],
                                     start=(ch == 0), stop=(ch == 1))
                nc.vector.tensor_scalar(out=fbias[:, oh:oh + 1], in0=pp[:, :],
                                        scalar1=s1[:, oh:oh + 1],
                                        scalar2=t1[:, oh:oh + 1],
                                        op0=ALU.mult, op1=ALU.add)

            bb = big.tile([128, 4, 2, PXC], F32)

            # ---------------- branches ----------------
            for br in range(4):
                k, pad, KK = KS[br], PADS[br], KKS[br]
                T = k * k * 2

                cby = mathp.tile([KK, PXC], F32, tag="cby")
                cbx = mathp.tile([KK, PXC], F32, tag="cbx")
                nc.sync.dma_start(cby[:, :], ins[f"cby{br}"][:, :])
                nc.sync.dma_start(cbx[:, :], ins[f"cbx{br}"][:, :])
                bof = mathp.tile([KK, 3], F32, tag="bof")
                nc.sync.dma_start(bof[:, :], ins[f"bof{br}"][:, :])

                psY = ps.tile([KK, PXC], F32, tag="A")
                psX = ps.tile([KK, PXC], F32, tag="B")
                psM = ps.tile([KK, PXC], F32, tag="C")
                for t in range(T):
                    h = t % 2
                    sp = t // 2
                    dy, dx = sp // k, sp % k
                    wof = wload.tile([128, 3 * KK], F32, tag="wof")
                    nc.sync.dma_start(wof[:, :], ins[f"wof{br}"][t, :, :])
                    ro = dy - pad + MARG
                    co = dx - pad + MARG
                    for nchk in range(2):
                        rview = xl[:, h, ro + 8 * nchk: ro + 8 * nchk + 8, co: co + W]
                        nsl = slice(512 * nchk, 512 * (nchk + 1))
                        st = (t == 0)
                        sp_ = (t == T - 1)
                        nc.tensor.matmul(psY[:, nsl], wof[:, 0:KK], rview,
                                         start=st, stop=sp_)
                        nc.tensor.matmul(psX[:, nsl], wof[:, KK:2 * KK], rview,
                                         start=st, stop=sp_)
                        nc.tensor.matmul(psM[:, nsl], wof[:, 2 * KK:3 * KK], rview,
                                         start=st, stop=sp_)

                # ---- per-pixel math (scratch slots s0..s7) ----
                def mt(tag):
                    return mathp.tile([KK, PXC], F32, tag=tag, name=tag)

                def TT(out, in0, in1, op):
                    nc.vector.tensor_tensor(out=out, in0=in0, in1=in1, op=op)

                def TS2(out, in0, a, bb_, op0, op1):
                    nc.vector.tensor_scalar(out=out, in0=in0, scalar1=a,
                                            scalar2=bb_, op0=op0, op1=op1)

                s0 = mt("s0"); s1t = mt("s1t"); s2 = mt("s2"); s3 = mt("s3")
                s4 = mt("s4"); s5t = mt("s5t"); s6 = mt("s6"); s7 = mt("s7")
                zi = mathp.tile([KK, PXC], I32, tag="zi")
                cw = mathp.tile([KK, 4, PXC], F32, tag="cw")
                qi16 = mathp.tile([KK, PXC], I16, tag="qi16")

                # pyY -> s0 ; pyX -> s1t
                TT(s0[:, :], psY[:, :], cby[:, :], ALU.add)
                nc.vector.tensor_scalar_add(s0[:, :], s0[:, :], bof[:, 0:1])
                TT(s1t[:, :], psX[:, :], cbx[:, :], ALU.add)
                nc.vector.tensor_scalar_add(s1t[:, :], s1t[:, :], bof[:, 1:2])
                # mod -> s2
                nc.scalar.activation(s2[:, :], psM[:, :], ACTF.Sigmoid,
                                     bias=bof[:, 2:3], scale=1.0)
                # y0 -> s3 (floor of s0)
                nc.vector.tensor_copy(zi[:, :], s0[:, :])
                nc.vector.tensor_copy(s3[:, :], zi[:, :])
                TT(s4[:, :], s3[:, :], s0[:, :], ALU.is_gt)
                TT(s3[:, :], s3[:, :], s4[:, :], ALU.subtract)
                # fy -> s4
                TT(s4[:, :], s0[:, :], s3[:, :], ALU.subtract)
                # x0 -> s0 (floor of s1t)
                nc.vector.tensor_copy(zi[:, :], s1t[:, :])
                nc.vector.tensor_copy(s0[:, :], zi[:, :])
                TT(s5t[:, :], s0[:, :], s1t[:, :], ALU.is_gt)
                TT(s0[:, :], s0[:, :], s5t[:, :], ALU.subtract)
                # fx -> s5t
                TT(s5t[:, :], s1t[:, :], s0[:, :], ALU.subtract)
                # q00 -> qi16 (uses s1t, s6 as scratch)
                TS2(s1t[:, :], s3[:, :], -1.0, 64.0, ALU.max, ALU.min)
                TS2(s6[:, :], s0[:, :], -1.0, 64.0, ALU.max, ALU.min)
                TS2(s1t[:, :], s1t[:, :], float(GW), float(MARG * GW + MARG),
                    ALU.mult, ALU.add)
                TT(s1t[:, :], s1t[:, :], s6[:, :], ALU.add)
                nc.vector.tensor_copy(qi16[:, :], s1t[:, :])
                q_t = dscr.tile([KK, PXC], I16, tag="q_t")
                nc.sync.dma_start(q_t[:, :], qi16[:, :])
                # vy0 -> s6 ; a0 = (1-fy)*vy0*mod -> s6
                TS2(s1t[:, :], s3[:, :], 0.0, 63.0, ALU.max, ALU.min)
                TT(s6[:, :], s1t[:, :], s3[:, :], ALU.is_equal)
                TS2(s1t[:, :], s4[:, :], -1.0, 1.0, ALU.mult, ALU.add)
                TT(s6[:, :], s6[:, :], s1t[:, :], ALU.mult)
                TT(s6[:, :], s6[:, :], s2[:, :], ALU.mult)
                # vy1 -> s7 ; a1 = fy*vy1*mod -> s7
                TS2(s1t[:, :], s3[:, :], -1.0, 62.0, ALU.max, ALU.min)
                TT(s7[:, :], s1t[:, :], s3[:, :], ALU.is_equal)
                TT(s7[:, :], s7[:, :], s4[:, :], ALU.mult)
                TT(s7[:, :], s7[:, :], s2[:, :], ALU.mult)
                # vx0 -> s4 ; b0 = (1-fx)*vx0 -> s4
                TS2(s1t[:, :], s0[:, :], 0.0, 63.0, ALU.max, ALU.min)
                TT(s4[:, :], s1t[:, :], s0[:, :], ALU.is_equal)
                TS2(s1t[:, :], s5t[:, :], -1.0, 1.0, ALU.mult, ALU.add)
                TT(s4[:, :], s4[:, :], s1t[:, :], ALU.mult)
                # vx1 -> s2 ; b1 = fx*vx1 -> s5t
                TS2(s1t[:, :], s0[:, :], -1.0, 62.0, ALU.max, ALU.min)
                TT(s2[:, :], s1t[:, :], s0[:, :], ALU.is_equal)
                TT(s5t[:, :], s5t[:, :], s2[:, :], ALU.mult)
                # corner weights
                TT(cw[:, 0, :], s6[:, :], s4[:, :], ALU.mult)
                TT(cw[:, 1, :], s6[:, :], s5t[:, :], ALU.mult)
                TT(cw[:, 2, :], s7[:, :], s4[:, :], ALU.mult)
                TT(cw[:, 3, :], s7[:, :], s5t[:, :], ALU.mult)
                w_t = dscr.tile([KK, 4, PXC], F32, tag="w_t")
                nc.sync.dma_start(w_t[:, :, :], cw[:, :, :])

                # ---- kk loop: gather + combine + matmul ----
                psO = (ps.tile([128, PXC], F32, tag="A", name="psO0"),
                       ps.tile([128, PXC], F32, tag="B", name="psO1"))
                for kk in range(KK):
                    it0 = gath.tile([128, 64], I16, tag="it0")
                    nc.sync.dma_start(
                        it0[0:16, :],
                        q_t[kk, :].rearrange("(s r) -> r s", r=16))
                    kr = 16
                    while kr < 128:
                        nc.sync.dma_start(it0[kr:2 * kr, :], it0[0:kr, :])
                        kr *= 2
                    it1 = gath.tile([128, 64], I16, tag="it1")
                    it2 = gath.tile([128, 64], I16, tag="it2")
                    it3 = gath.tile([128, 64], I16, tag="it3")
                    nc.vector.tensor_scalar_add(it1[:, :], it0[:, :], 1)
                    nc.vector.tensor_scalar_add(it2[:, :], it0[:, :], GW)
                    nc.vector.tensor_scalar_add(it3[:, :], it0[:, :], GW + 1)
                    its = (it0, it1, it2, it3)
                    wrs = []
                    for j in range(4):
                        wr = wrp.tile([128, PXC], F32, tag=f"wr{j}")
                        nc.sync.dma_start(
                            wr[:, :],
                            w_t[kk, j, :].unsqueeze(0).broadcast_to([128, PXC]))
                        wrs.append(wr)
                    for h in range(2):
                        acc = gath.tile([128, PXC], F32, tag="acc")
                        g = gath.tile([128, PXC], F32, tag="g")
                        for j in range(4):
                            nc.gpsimd.ap_gather(
                                g[:, :], xp[:, h, :], its[j][:, :],
                                channels=128, num_elems=GNP, d=1, num_idxs=PXC)
                            if j == 0:
                                TT(acc[:, :], g[:, :], wrs[j][:, :], ALU.mult)
                            else:
                                TT(g[:, :], g[:, :], wrs[j][:, :], ALU.mult)
                                TT(acc[:, :], acc[:, :], g[:, :], ALU.add)
                            if j < 3:
                                g = gath.tile([128, PXC], F32, tag="g")
                        for oh in range(2):
                            wm = wload.tile([128, 128], F32, tag="wm")
                            nc.sync.dma_start(wm[:, :],
                                              ins[f"wm{br}"][kk, h, oh, :, :])
                            first = (kk == 0 and h == 0)
                            last = (kk == KK - 1 and h == 1)
                            for nchk in range(2):
                                nsl = slice(512 * nchk, 512 * (nchk + 1))
                                nc.tensor.matmul(psO[oh][:, nsl], wm[:, :],
                                                 acc[:, nsl],
                                                 start=first, stop=last)

                sbn = mathp.tile([128, 2], F32, tag="sbn")
                tbn = mathp.tile([128, 2], F32, tag="tbn")
                nc.sync.dma_start(sbn[:, :], ins[f"sbn{br}"][:, :])
                nc.sync.dma_start(tbn[:, :], ins[f"tbn{br}"][:, :])
                for oh in range(2):
                    nc.scalar.activation(bb[:, br, oh, :], psO[oh][:, :],
                                         ACTF.Relu, bias=tbn[:, oh:oh + 1],
                                         scale=sbn[:, oh:oh + 1])

            # ---------------- fuse ----------------
            wfu = big.tile([128, 8, 2, 128], F32)
            for ch in range(8):
                nc.sync.dma_start(wfu[:, ch, :, :], wfu_d[ch, :, :, :])
            out_t = big.tile([128, 2, PXC], F32)
            for oh in range(2):
                psF = ps.tile([128, PXC], F32, tag="A")
                for ch in range(8):
                    brc, hb = ch // 2, ch % 2
                    for nchk in range(2):
                        nsl = slice(512 * nchk, 512 * (nchk + 1))
                        nc.tensor.matmul(psF[:, nsl], wfu[:, ch, oh, :],
                                         bb[:, brc, hb, nsl],
                                         start=(ch == 0), stop=(ch == 7))
                nc.scalar.activation(out_t[:, oh, :], psF[:, :], ACTF.Relu,
                                     bias=fbias[:, oh:oh + 1],
                                     scale=s1[:, oh:oh + 1])
            nc.sync.dma_start(outp_d[:, :, :].rearrange("t o p -> o t p"),
                              out_t[:, :, :])

    nc.compile()
    return nc


# ---------------------------------------------------------------------------
# host-side preparation
# ---------------------------------------------------------------------------

def _prep_shared(branch_params, pool_params, fuse_params):
    sh = {}
    for br in range(4):
        k, KK = KS[br], KKS[br]
        off_w, off_b, mod_w, mod_b, w, g, b, m, v = [np.asarray(t, np.float32)
                                                     for t in branch_params[br]]
        T = k * k * 2
        wof = np.zeros((T, 128, 3 * KK), np.float32)
        offY = off_w[0::2]
        offX = off_w[1::2]
        for sp in range(k * k):
            dy, dx = sp // k, sp % k
            for h in range(2):
                t = sp * 2 + h
                cs = slice(128 * h, 128 * (h + 1))
                wof[t, :, 0:KK] = offY[:, cs, dy, dx].T
                wof[t, :, KK:2 * KK] = offX[:, cs, dy, dx].T
                wof[t, :, 2 * KK:3 * KK] = mod_w[:, cs, dy, dx].T
        sh[f"wof{br}"] = wof
        bof = np.zeros((KK, 3), np.float32)
        bof[:, 0] = off_b[0::2]
        bof[:, 1] = off_b[1::2]
        bof[:, 2] = mod_b
        sh[f"bof{br}"] = bof
        wr = (2.0 * w).reshape(256, 256, KK).astype(np.float32)
        wm = np.zeros((KK, 2, 2, 128, 128), np.float32)
        for kk in range(KK):
            for h in range(2):
                for oh in range(2):
                    wm[kk, h, oh] = wr[128 * oh:128 * (oh + 1),
                                       128 * h:128 * (h + 1), kk].T
        sh[f"wm{br}"] = wm
        s, t = _bn_fold(g, b, m, v)
        sh[f"sbn{br}"] = np.ascontiguousarray(s.reshape(2, 128).T)
        sh[f"tbn{br}"] = np.ascontiguousarray(t.reshape(2, 128).T)

    pw, pg, pb, pm, pv = [np.asarray(t, np.float32) for t in pool_params]
    pw2 = pw.reshape(256, 256)
    wpl = np.zeros((2, 128, 2, 128), np.float32)
    for ch in range(2):
        for oh in range(2):
            wpl[ch, :, oh, :] = pw2[128 * oh:128 * (oh + 1),
                                    128 * ch:128 * (ch + 1)].T
    sh["wpl"] = wpl
    s5, t5 = _bn_fold(pg, pb, pm, pv)
    sh["s5"] = np.ascontiguousarray(s5.reshape(2, 128).T)
    sh["t5"] = np.ascontiguousarray(t5.reshape(2, 128).T)

    w1, g1, b1, m1, v1 = [np.asarray(t, np.float32) for t in fuse_params]
    w1r = w1.reshape(256, 1280)
    wfu = np.zeros((8, 128, 2, 128), np.float32)
    for ch in range(8):
        for oh in range(2):
            wfu[ch, :, oh, :] = w1r[128 * oh:128 * (oh + 1),
                                    128 * ch:128 * (ch + 1)].T
    sh["wfu"] = wfu
    wfp = np.zeros((2, 128, 2, 128), np.float32)
    for ch in range(2):
        for oh in range(2):
            wfp[ch, :, oh, :] = w1r[128 * oh:128 * (oh + 1),
                                    1024 + 128 * ch:1024 + 128 * (ch + 1)].T
    sh["wfp"] = wfp
    s1, t1 = _bn_fold(g1, b1, m1, v1)
    sh["s1"] = np.ascontiguousarray(s1.reshape(2, 128).T)
    sh["t1"] = np.ascontiguousarray(t1.reshape(2, 128).T)
    return sh


def _prep_core(x, core):
    bidx, rb = core // 4, core % 4
    m = {}
    grid = np.zeros((256, GW, GW), np.float32)
    grid[:, MARG:MARG + H, MARG:MARG + W] = x[bidx]
    flat = np.zeros((256, GNP), np.float32)
    flat[:, :GN] = grid.reshape(256, GN)
    m["xp"] = np.ascontiguousarray(flat.reshape(2, 128, GNP).transpose(1, 0, 2))
    slab = grid[:, 16 * rb:16 * rb + 22, :]          # rows 16rb-3 .. 16rb+18
    m["xl"] = np.ascontiguousarray(slab.reshape(2, 128, 22, GW).transpose(1, 0, 2, 3))
    ii, jj = np.meshgrid(np.arange(RB) + 16 * rb, np.arange(W), indexing="ij")
    ii = ii.reshape(-1).astype(np.float32)
    jj = jj.reshape(-1).astype(np.float32)
    for br in range(4):
        k, pad, KK = KS[br], PADS[br], KKS[br]
        gi, gj = np.meshgrid(np.arange(k), np.arange(k), indexing="ij")
        gi = gi.reshape(-1).astype(np.float32)
        gj = gj.reshape(-1).astype(np.float32)
        m[f"cby{br}"] = (ii[None, :] - pad) + gi[:, None]
        m[f"cbx{br}"] = (jj[None, :] - pad) + gj[:, None]
    return m


def make_in_maps(x, branch_params, pool_params, fuse_params):
    x = np.asarray(x, np.float32)
    sh = _prep_shared(branch_params, pool_params, fuse_params)
    in_maps = []
    for core in range(NCORES):
        m = dict(sh)
        m.update(_prep_core(x, core))
        m = {k2: np.ascontiguousarray(v) for k2, v in m.items()}
        in_maps.append(m)
    return in_maps


def stitch(results):
    out = np.zeros((B, 256, H, W), np.float32)
    for core in range(NCORES):
        bidx, rb = core // 4, core % 4
        o = results[core]["outp"].reshape(2, 128, RB, W)
        out[bidx, 0:128, 16 * rb:16 * rb + RB, :] = o[0]
        out[bidx, 128:256, 16 * rb:16 * rb + RB, :] = o[1]
    return out


_PROGRAM = None


def kernel(x, branch_params, pool_params, fuse_params):
    global _PROGRAM
    if _PROGRAM is None:
        _PROGRAM = build_program()
    in_maps = make_in_maps(x, branch_params, pool_params, fuse_params)
    res = run_bass_kernel_spmd(_PROGRAM, in_maps, core_ids=list(range(NCORES)))
    return stitch(res.results)
